# revision 3
# baseline (speedup 1.0000x reference)
"""Multi-head attention (B=2, S=2048, D=1024, H=16, dk=64) on 8 Trainium2
NeuronCores via Bass/Tile.

Sharding: core c handles batch b = c//4 and head-group g = c%4 (4 heads,
256 qkv columns).  Each core computes its QKV projection slices, 4 heads of
attention, and a partial output projection against its 256-row slice of Wo.
The host sums the 4 partial outputs per batch (row-sharded Wo => partial
sums) and folds in the biases bo and bv@Wo (softmax rows sum to 1, so the
V-bias contributes exactly bv@Wo per token).

v3 design notes (vs v2 baseline at 378us):
- All matmuls bf16 (fp32 HIGH mode triggered the activity power throttle:
  50% PE util limit for 60% of runtime; bf16 runs 1 cyc/row at any free
  size).  PSUM accumulation stays fp32; softmax denominators stay fp32
  through the reciprocal path; num/denom share the same bf16 exp values so
  normalization error largely cancels.
- Host pre-transposes x and pre-casts/pre-arranges all weights into the
  exact SBUF layouts, removing the on-device DMA transposes, hi/lo split
  adds and fp32->f32r casts that serialized the first 42us.
- Weight DMAs ride the scalar-engine HWDGE queue, x/out DMAs the sync
  queue, so startup transfers overlap.
- Scores land in one shared 4-bank PSUM tensor [128, 4, 512]; ONE ACT exp
  per 2 k-chunks covers [128, 1024] and writes bf16 directly (AV rhs).
- AV lhsT = [V_h | 1] so PSUM row 64 accumulates the softmax denominators.
- Normalization: DVE reciprocal of the fp32 sums row, PE ones
  outer-product broadcast (f32r, N=512 => full rate), DVE multiply to
  bf16, then a partition-shifting SBUF->SBUF DMA routes heads into O^T.
"""

import numpy as np

P = 128
B, S, D = 2, 2048, 1024
H, DK = 16, 64
COLS = 256          # qkv columns per core (4 heads)
KC = D // P         # 8 contraction chunks for the projections
TT = 512            # token block (matmul free dim)
NJ = S // TT        # 4 token blocks
NT = S // P         # 16 token tiles
NKT = S // P        # 16 key tiles
VW = 65             # per-head AV lhsT width: ones column + 64 v-dims

_CACHE = {}


def _build():
    import concourse.bass as bass
    import concourse.tile as tile
    from concourse import bacc, mybir

    f32 = mybir.dt.float32
    f32r = mybir.dt.float32r
    bf16 = mybir.dt.bfloat16
    Exp = mybir.ActivationFunctionType.Exp

    nc = bacc.Bacc(
        "TRN2", target_bir_lowering=False, debug=False,
        enable_asserts=False, num_devices=8,
    )
    xt_d = nc.dram_tensor("xt", [P, KC, S], bf16, kind="ExternalInput").ap()
    wq_d = nc.dram_tensor("wq", [P, KC, COLS], bf16, kind="ExternalInput").ap()
    wk_d = nc.dram_tensor("wk", [P, KC, COLS], bf16, kind="ExternalInput").ap()
    wv_d = nc.dram_tensor("wv", [P, KC, COLS], bf16, kind="ExternalInput").ap()
    wo_d = nc.dram_tensor("wo", [P, 2, D], bf16, kind="ExternalInput").ap()
    bq_d = nc.dram_tensor("bq", [P, 2], f32, kind="ExternalInput").ap()
    bk_d = nc.dram_tensor("bk", [P, 2], f32, kind="ExternalInput").ap()
    out_d = nc.dram_tensor("out_t", [D, S], f32, kind="ExternalOutput").ap()

    with tile.TileContext(nc) as tc:
        with (
            tc.tile_pool(name="const", bufs=1) as const,
            tc.tile_pool(name="wpool", bufs=1) as wpool,
            tc.tile_pool(name="persist", bufs=1) as persist,
            tc.tile_pool(name="xtp", bufs=2) as xtp,
            tc.tile_pool(name="exps", bufs=3) as exps,
            tc.tile_pool(name="stage", bufs=3) as stage,
            tc.tile_pool(name="outst", bufs=4) as outst,
            tc.tile_pool(name="ps_sc", bufs=1, space="PSUM") as ps_sc,
            tc.tile_pool(name="ps_acc", bufs=2, space="PSUM") as ps_acc,
            tc.tile_pool(name="ps_u", bufs=2, space="PSUM") as ps_u,
        ):
            ones_bf = const.tile([P, VW], bf16, tag="ones_bf")
            nc.vector.memset(ones_bf[:], 1.0)
            ones_f = const.tile([1, 64], f32, tag="ones_f")
            nc.vector.memset(ones_f[:], 1.0)
            ones_r = const.tile([1, 64], f32r, tag="ones_r")
            nc.vector.tensor_copy(ones_r[:], ones_f[:])

            # weights arrive pre-arranged/pre-cast; scalar-engine DMA queue
            wq_sb = wpool.tile([P, KC, COLS], bf16, tag="w_q")
            nc.scalar.dma_start(wq_sb[:], wq_d)
            wk_sb = wpool.tile([P, KC, COLS], bf16, tag="w_k")
            nc.scalar.dma_start(wk_sb[:], wk_d)
            wv_sb = wpool.tile([P, KC, COLS], bf16, tag="w_v")
            nc.scalar.dma_start(wv_sb[:], wv_d)
            wo_sb = wpool.tile([P, 2, D], bf16, tag="w_o")
            nc.scalar.dma_start(wo_sb[:], wo_d)
            bq_sb = const.tile([P, 2], f32, tag="bq")
            nc.scalar.dma_start(bq_sb[:], bq_d)
            bk_sb = const.tile([P, 2], f32, tag="bk")
            nc.scalar.dma_start(bk_sb[:], bk_d)

            # persistent activations (all bf16)
            qT = persist.tile([P, 2, S], bf16, tag="qT")    # [qcol, tok]
            kT = persist.tile([P, 2, S], bf16, tag="kT")    # [kcol, tok]
            vt = persist.tile([P, NT, 4 * VW], bf16, tag="vt")  # [tok, h*(V|1)]
            oT = persist.tile([P, 2, S], bf16, tag="oT")    # [vdim, tok]

            # ones column (index 64 of each head's VW slice)
            vt_heads = vt[:].rearrange("p t (h c) -> p t h c", c=VW)
            nc.vector.tensor_copy(
                vt_heads[:, :, :, 64],
                ones_bf[:, :NT * 4].rearrange("p (t h) -> p t h", h=4),
            )

            # ---- phase 1: QKV projections, per token block ----
            for j in range(NJ):
                xT = xtp.tile([P, KC, TT], bf16, tag="xT")
                nc.sync.dma_start(xT[:], xt_d[:, :, bass.ts(j, TT)])

                # Q^T, K^T: [qcol, tok] with bias
                for (wmat, bsb, dstT) in ((wq_sb, bq_sb, qT), (wk_sb, bk_sb, kT)):
                    for ct in range(2):
                        acc = ps_u.tile([P, TT], f32, tag="u", name="qk_acc")
                        for kc in range(KC):
                            nc.tensor.matmul(
                                acc[:], wmat[:, kc, bass.ts(ct, P)], xT[:, kc, :],
                                start=(kc == 0), stop=(kc == KC - 1),
                            )
                        nc.vector.tensor_scalar_add(
                            dstT[:, ct, bass.ts(j, TT)], acc[:], bsb[:, ct : ct + 1]
                        )

                # V: [tok, vcol]
                for ts in range(TT // P):
                    acc = ps_u.tile([P, COLS], f32, tag="u", name="v_acc")
                    for kc in range(KC):
                        nc.tensor.matmul(
                            acc[:], xT[:, kc, bass.ts(ts, P)], wv_sb[:, kc, :],
                            start=(kc == 0), stop=(kc == KC - 1),
                        )
                    tt = 4 * j + ts
                    nc.vector.tensor_copy(
                        vt_heads[:, tt, :, 0:64],
                        acc[:].rearrange("p (h c) -> p h c", c=64),
                    )

            # shared scores PSUM tensor: 4 slots x [128, 512] = 4 banks
            big_sc = ps_sc.tile([P, 4, TT], f32, tag="sc")

            # ---- phase 2 + 3 interleaved over token blocks ----
            for j in range(NJ):
                for p in range(2):
                    o_ps = [
                        ps_acc.tile([VW, TT], f32, tag="acc", name=f"o_ps{i}")
                        for i in range(2)
                    ]
                    # software-pipelined emission: scores run 2 k-chunks ahead,
                    # AV trails exp by one, so PE always has ready work while
                    # ACT's exp latency is in flight.
                    def sc_emit(kc):
                        base = (2 * kc) % 4
                        for i in range(2):
                            lo, hi = 64 * i, 64 * i + 64
                            nc.tensor.matmul(
                                big_sc[:, base + i, :],
                                kT[lo:hi, p, bass.ts(kc, P)],
                                qT[lo:hi, p, bass.ts(j, TT)],
                                start=True, stop=True,
                            )

                    def av_emit(kc, ex):
                        for i in range(2):
                            h = 2 * p + i
                            nc.tensor.matmul(
                                o_ps[i][:],
                                vt[:, kc, bass.ds(VW * h, VW)],
                                ex[:, i, :],
                                start=(kc == 0), stop=(kc == NKT - 1),
                            )

                    sc_emit(0)
                    sc_emit(1)
                    prev = None
                    for kc in range(NKT):
                        base = (2 * kc) % 4
                        ex = exps.tile([P, 2, TT], bf16, tag="exp", name="ex")
                        nc.scalar.activation(
                            ex[:], big_sc[:, base : base + 2, :], Exp,
                            scale=0.125,
                        )
                        if prev is not None:
                            av_emit(kc - 1, prev)
                        if kc + 2 < NKT:
                            sc_emit(kc + 2)
                        prev = ex
                    av_emit(NKT - 1, prev)

                    # normalize both heads into O^T: fp32 denominators,
                    # PE ones outer-product broadcast, bf16 result
                    o32 = stage.tile([P, TT], bf16, tag="o32", name="o32")
                    for i in range(2):
                        osb = stage.tile([P, TT], bf16, tag="osb", name="osb")
                        nc.vector.tensor_copy(osb[0:VW, :], o_ps[i][:])
                        srow = stage.tile([1, TT], f32r, tag="srow", name="srow")
                        nc.vector.tensor_copy(srow[:], o_ps[i][64:65, :])
                        rbc = ps_u.tile([64, TT], f32, tag="u", name="rbc")
                        nc.tensor.matmul(
                            rbc[:], ones_r[0:1, :], srow[0:1, :],
                            start=True, stop=True,
                        )
                        rbs = stage.tile([64, TT], f32, tag="rbs", name="rbs")
                        nc.vector.reciprocal_approx_fast(rbs[:], rbc[:])
                        onrm = stage.tile([P, TT], bf16, tag="onrm", name="onrm")
                        nc.vector.tensor_tensor(
                            onrm[0:64, :], osb[0:64, :], rbs[:],
                            mybir.AluOpType.mult,
                        )
                        nc.sync.dma_start(
                            o32[bass.ds(64 * i, 64), :], onrm[0:64, :]
                        )
                    nc.vector.tensor_copy(oT[:, p, bass.ts(j, TT)], o32[:])

                # partial output projection for this token block
                for oc in range(D // P):
                    acc = ps_u.tile([P, TT], f32, tag="u", name="wo_acc")
                    for vc in range(2):
                        nc.tensor.matmul(
                            acc[:], wo_sb[:, vc, bass.ts(oc, P)],
                            oT[:, vc, bass.ts(j, TT)],
                            start=(vc == 0), stop=(vc == 1),
                        )
                    st = outst.tile([P, TT], f32, tag="outst", name="outst")
                    nc.vector.tensor_copy(st[:], acc[:])
                    nc.sync.dma_start(out_d[bass.ts(oc, P), bass.ts(j, TT)], st[:])

    nc.compile()
    return nc


def make_in_maps(x, Wq, bq, Wk, bk, Wv, Wo):
    import ml_dtypes

    bf = ml_dtypes.bfloat16

    def shard3(a, o):  # [o*P, f] -> [P, o, f]
        return np.ascontiguousarray(
            a.reshape(o, P, a.shape[1]).transpose(1, 0, 2)
        )

    xt = [
        shard3(np.ascontiguousarray(x[b].T).astype(bf), KC) for b in range(B)
    ]

    in_maps = []
    for c in range(8):
        b, g = divmod(c, 4)
        cs = slice(COLS * g, COLS * (g + 1))
        in_maps.append({
            "xt": xt[b],
            "wq": shard3(Wq[:, cs].astype(bf), KC),
            "wk": shard3(Wk[:, cs].astype(bf), KC),
            "wv": shard3(Wv[:, cs].astype(bf), KC),
            "wo": shard3(Wo[cs, :].astype(bf), 2),
            "bq": np.ascontiguousarray(bq[cs].reshape(2, P).T),
            "bk": np.ascontiguousarray(bk[cs].reshape(2, P).T),
        })
    return in_maps


def kernel(x, Wq, bq, Wk, bk, Wv, bv, Wo, bo):
    from concourse import bass_utils

    x = np.asarray(x, dtype=np.float32)
    Wq = np.asarray(Wq, dtype=np.float32)
    Wk = np.asarray(Wk, dtype=np.float32)
    Wv = np.asarray(Wv, dtype=np.float32)
    Wo = np.asarray(Wo, dtype=np.float32)
    bq = np.asarray(bq, dtype=np.float32)
    bk = np.asarray(bk, dtype=np.float32)
    bv = np.asarray(bv, dtype=np.float32)
    bo = np.asarray(bo, dtype=np.float32)

    if "nc" not in _CACHE:
        _CACHE["nc"] = _build()
    nc = _CACHE["nc"]

    in_maps = make_in_maps(x, Wq, bq, Wk, bk, Wv, Wo)
    res = bass_utils.run_bass_kernel_spmd(nc, in_maps, core_ids=list(range(8)))

    out = np.zeros((B, S, D), dtype=np.float32)
    for c in range(8):
        out[c // 4] += res.results[c]["out_t"].T
    out += bo + bv @ Wo
    return out


# revision 4
# speedup vs baseline: 1.2235x; 1.2235x over previous
"""Multi-head attention (B=2, S=2048, D=1024, H=16, dk=64) on 8 Trainium2
NeuronCores via Bass/Tile.

Sharding: core c handles batch b = c//4 and head-group g = c%4 (4 heads,
256 qkv columns).  Each core computes its QKV projection slices, 4 heads of
attention, and a partial output projection against its 256-row slice of Wo.
The host sums the 4 partial outputs per batch (row-sharded Wo => partial
sums) and folds in the biases bo and bv@Wo (softmax rows sum to 1, so the
V-bias contributes exactly bv@Wo per token).

v3 design notes (vs v2 baseline at 378us):
- All matmuls bf16 (fp32 HIGH mode triggered the activity power throttle:
  50% PE util limit for 60% of runtime; bf16 runs 1 cyc/row at any free
  size).  PSUM accumulation stays fp32; softmax denominators stay fp32
  through the reciprocal path; num/denom share the same bf16 exp values so
  normalization error largely cancels.
- Host pre-transposes x and pre-casts/pre-arranges all weights into the
  exact SBUF layouts, removing the on-device DMA transposes, hi/lo split
  adds and fp32->f32r casts that serialized the first 42us.
- Weight DMAs ride the scalar-engine HWDGE queue, x/out DMAs the sync
  queue, so startup transfers overlap.
- Scores land in one shared 4-bank PSUM tensor [128, 4, 512]; ONE ACT exp
  per 2 k-chunks covers [128, 1024] and writes bf16 directly (AV rhs).
- AV lhsT = [V_h | 1] so PSUM row 64 accumulates the softmax denominators.
- Normalization: DVE reciprocal of the fp32 sums row, PE ones
  outer-product broadcast (f32r, N=512 => full rate), DVE multiply to
  bf16, then a partition-shifting SBUF->SBUF DMA routes heads into O^T.
"""

import numpy as np

P = 128
B, S, D = 2, 2048, 1024
H, DK = 16, 64
COLS = 256          # qkv columns per core (4 heads)
KC = D // P         # 8 contraction chunks for the projections
TT = 512            # token block (matmul free dim)
NJ = S // TT        # 4 token blocks
NT = S // P         # 16 token tiles
NKT = S // P        # 16 key tiles
VW = 65             # per-head AV lhsT width: ones column + 64 v-dims

_CACHE = {}


def _build():
    import concourse.bass as bass
    import concourse.tile as tile
    from concourse import bacc, mybir

    f32 = mybir.dt.float32
    f32r = mybir.dt.float32r
    bf16 = mybir.dt.bfloat16
    Exp = mybir.ActivationFunctionType.Exp

    nc = bacc.Bacc(
        "TRN2", target_bir_lowering=False, debug=False,
        enable_asserts=False, num_devices=8,
    )
    xt_d = nc.dram_tensor("xt", [P, KC, S], bf16, kind="ExternalInput").ap()
    wq_d = nc.dram_tensor("wq", [P, KC, COLS], bf16, kind="ExternalInput").ap()
    wk_d = nc.dram_tensor("wk", [P, KC, COLS], bf16, kind="ExternalInput").ap()
    wv_d = nc.dram_tensor("wv", [P, KC, COLS], bf16, kind="ExternalInput").ap()
    wo_d = nc.dram_tensor("wo", [P, 2, D], bf16, kind="ExternalInput").ap()
    bq_d = nc.dram_tensor("bq", [P, 2], f32, kind="ExternalInput").ap()
    bk_d = nc.dram_tensor("bk", [P, 2], f32, kind="ExternalInput").ap()
    out_d = nc.dram_tensor("out_t", [D, S], f32, kind="ExternalOutput").ap()

    with tile.TileContext(nc) as tc:
        with (
            tc.tile_pool(name="const", bufs=1) as const,
            tc.tile_pool(name="wpool", bufs=1) as wpool,
            tc.tile_pool(name="persist", bufs=1) as persist,
            tc.tile_pool(name="xtp", bufs=2) as xtp,
            tc.tile_pool(name="exps", bufs=3) as exps,
            tc.tile_pool(name="stage", bufs=3) as stage,
            tc.tile_pool(name="outst", bufs=4) as outst,
            tc.tile_pool(name="ps_sc", bufs=1, space="PSUM") as ps_sc,
            tc.tile_pool(name="ps_acc", bufs=2, space="PSUM") as ps_acc,
            tc.tile_pool(name="ps_u", bufs=2, space="PSUM") as ps_u,
        ):
            ones_bf = const.tile([P, VW], bf16, tag="ones_bf")
            nc.vector.memset(ones_bf[:], 1.0)
            ones_f = const.tile([1, 64], f32, tag="ones_f")
            nc.vector.memset(ones_f[:], 1.0)
            ones_r = const.tile([1, 64], f32r, tag="ones_r")
            nc.vector.tensor_copy(ones_r[:], ones_f[:])

            # weights arrive pre-arranged/pre-cast; scalar-engine DMA queue
            wq_sb = wpool.tile([P, KC, COLS], bf16, tag="w_q")
            nc.scalar.dma_start(wq_sb[:], wq_d)
            wk_sb = wpool.tile([P, KC, COLS], bf16, tag="w_k")
            nc.scalar.dma_start(wk_sb[:], wk_d)
            wv_sb = wpool.tile([P, KC, COLS], bf16, tag="w_v")
            nc.scalar.dma_start(wv_sb[:], wv_d)
            wo_sb = wpool.tile([P, 2, D], bf16, tag="w_o")
            nc.scalar.dma_start(wo_sb[:], wo_d)
            bq_sb = const.tile([P, 2], f32, tag="bq")
            nc.scalar.dma_start(bq_sb[:], bq_d)
            bk_sb = const.tile([P, 2], f32, tag="bk")
            nc.scalar.dma_start(bk_sb[:], bk_d)

            # persistent activations (all bf16)
            qT = persist.tile([P, 2, S], bf16, tag="qT")    # [qcol, tok]
            kT = persist.tile([P, 2, S], bf16, tag="kT")    # [kcol, tok]
            vt = persist.tile([P, NT, 4 * VW], bf16, tag="vt")  # [tok, h*(V|1)]
            oT = persist.tile([P, 2, S], bf16, tag="oT")    # [vdim, tok]

            # ones column (index 64 of each head's VW slice)
            vt_heads = vt[:].rearrange("p t (h c) -> p t h c", c=VW)
            nc.vector.tensor_copy(
                vt_heads[:, :, :, 64],
                ones_bf[:, :NT * 4].rearrange("p (t h) -> p t h", h=4),
            )

            # ---- phase 1: QKV projections, per token block ----
            for j in range(NJ):
                xT = xtp.tile([P, KC, TT], bf16, tag="xT")
                nc.sync.dma_start(xT[:], xt_d[:, :, bass.ts(j, TT)])

                # Q^T, K^T: [qcol, tok] with bias
                for (wmat, bsb, dstT) in ((wq_sb, bq_sb, qT), (wk_sb, bk_sb, kT)):
                    for ct in range(2):
                        acc = ps_u.tile([P, TT], f32, tag="u", name="qk_acc")
                        for kc in range(KC):
                            nc.tensor.matmul(
                                acc[:], wmat[:, kc, bass.ts(ct, P)], xT[:, kc, :],
                                start=(kc == 0), stop=(kc == KC - 1),
                            )
                        nc.vector.tensor_scalar_add(
                            dstT[:, ct, bass.ts(j, TT)], acc[:], bsb[:, ct : ct + 1]
                        )

                # V: [tok, vcol]
                for ts in range(TT // P):
                    acc = ps_u.tile([P, COLS], f32, tag="u", name="v_acc")
                    for kc in range(KC):
                        nc.tensor.matmul(
                            acc[:], xT[:, kc, bass.ts(ts, P)], wv_sb[:, kc, :],
                            start=(kc == 0), stop=(kc == KC - 1),
                        )
                    tt = 4 * j + ts
                    nc.vector.tensor_copy(
                        vt_heads[:, tt, :, 0:64],
                        acc[:].rearrange("p (h c) -> p h c", c=64),
                    )

            # ---- phase 2 + 3 interleaved over token blocks ----
            for j in range(NJ):
                for p in range(2):
                    o_ps = [
                        ps_acc.tile([VW, TT], f32, tag="acc", name=f"o_ps{i}")
                        for i in range(2)
                    ]
                    # software-pipelined emission: scores run 2 k-chunks ahead,
                    # AV trails exp by one, so PE always has ready work while
                    # ACT's exp latency is in flight.  Each k-chunk's scores
                    # get their OWN double-buffered PSUM tile so the scheduler
                    # tracks exact deps (a single shared 4-bank tensor made
                    # every ACT wait on the latest scores matmul emitted
                    # before it, serializing ACT+scores at ~2us per chunk).
                    sc_tiles = {}

                    def sc_emit(kc):
                        sc = ps_sc.tile([P, 2, TT], f32, tag="sc", name="sc")
                        sc_tiles[kc] = sc
                        for i in range(2):
                            lo, hi = 64 * i, 64 * i + 64
                            nc.tensor.matmul(
                                sc[:, i, :],
                                kT[lo:hi, p, bass.ts(kc, P)],
                                qT[lo:hi, p, bass.ts(j, TT)],
                                start=True, stop=True,
                            )

                    def av_emit(kc, ex):
                        for i in range(2):
                            h = 2 * p + i
                            nc.tensor.matmul(
                                o_ps[i][:],
                                vt[:, kc, bass.ds(VW * h, VW)],
                                ex[:, i, :],
                                start=(kc == 0), stop=(kc == NKT - 1),
                            )

                    sc_emit(0)
                    sc_emit(1)
                    prev = None
                    for kc in range(NKT):
                        ex = exps.tile([P, 2, TT], bf16, tag="exp", name="ex")
                        nc.scalar.activation(
                            ex[:], sc_tiles.pop(kc)[:], Exp,
                            scale=0.125,
                        )
                        if prev is not None:
                            av_emit(kc - 1, prev)
                        if kc + 2 < NKT:
                            sc_emit(kc + 2)
                        prev = ex
                    av_emit(NKT - 1, prev)

                    # normalize both heads into O^T: fp32 denominators,
                    # PE ones outer-product broadcast, bf16 result
                    o32 = stage.tile([P, TT], bf16, tag="o32", name="o32")
                    for i in range(2):
                        osb = stage.tile([P, TT], bf16, tag="osb", name="osb")
                        nc.vector.tensor_copy(osb[0:VW, :], o_ps[i][:])
                        srow = stage.tile([1, TT], f32r, tag="srow", name="srow")
                        nc.vector.tensor_copy(srow[:], o_ps[i][64:65, :])
                        rbc = ps_u.tile([64, TT], f32, tag="u", name="rbc")
                        nc.tensor.matmul(
                            rbc[:], ones_r[0:1, :], srow[0:1, :],
                            start=True, stop=True,
                        )
                        rbs = stage.tile([64, TT], f32, tag="rbs", name="rbs")
                        nc.vector.reciprocal_approx_fast(rbs[:], rbc[:])
                        onrm = stage.tile([P, TT], bf16, tag="onrm", name="onrm")
                        nc.vector.tensor_tensor(
                            onrm[0:64, :], osb[0:64, :], rbs[:],
                            mybir.AluOpType.mult,
                        )
                        nc.sync.dma_start(
                            o32[bass.ds(64 * i, 64), :], onrm[0:64, :]
                        )
                    nc.vector.tensor_copy(oT[:, p, bass.ts(j, TT)], o32[:])

                # partial output projection for this token block
                for oc in range(D // P):
                    acc = ps_u.tile([P, TT], f32, tag="u", name="wo_acc")
                    for vc in range(2):
                        nc.tensor.matmul(
                            acc[:], wo_sb[:, vc, bass.ts(oc, P)],
                            oT[:, vc, bass.ts(j, TT)],
                            start=(vc == 0), stop=(vc == 1),
                        )
                    st = outst.tile([P, TT], f32, tag="outst", name="outst")
                    nc.vector.tensor_copy(st[:], acc[:])
                    nc.sync.dma_start(out_d[bass.ts(oc, P), bass.ts(j, TT)], st[:])

    nc.compile()
    return nc


def make_in_maps(x, Wq, bq, Wk, bk, Wv, Wo):
    import ml_dtypes

    bf = ml_dtypes.bfloat16

    def shard3(a, o):  # [o*P, f] -> [P, o, f]
        return np.ascontiguousarray(
            a.reshape(o, P, a.shape[1]).transpose(1, 0, 2)
        )

    xt = [
        shard3(np.ascontiguousarray(x[b].T).astype(bf), KC) for b in range(B)
    ]

    in_maps = []
    for c in range(8):
        b, g = divmod(c, 4)
        cs = slice(COLS * g, COLS * (g + 1))
        in_maps.append({
            "xt": xt[b],
            "wq": shard3(Wq[:, cs].astype(bf), KC),
            "wk": shard3(Wk[:, cs].astype(bf), KC),
            "wv": shard3(Wv[:, cs].astype(bf), KC),
            "wo": shard3(Wo[cs, :].astype(bf), 2),
            "bq": np.ascontiguousarray(bq[cs].reshape(2, P).T),
            "bk": np.ascontiguousarray(bk[cs].reshape(2, P).T),
        })
    return in_maps


def kernel(x, Wq, bq, Wk, bk, Wv, bv, Wo, bo):
    from concourse import bass_utils

    x = np.asarray(x, dtype=np.float32)
    Wq = np.asarray(Wq, dtype=np.float32)
    Wk = np.asarray(Wk, dtype=np.float32)
    Wv = np.asarray(Wv, dtype=np.float32)
    Wo = np.asarray(Wo, dtype=np.float32)
    bq = np.asarray(bq, dtype=np.float32)
    bk = np.asarray(bk, dtype=np.float32)
    bv = np.asarray(bv, dtype=np.float32)
    bo = np.asarray(bo, dtype=np.float32)

    if "nc" not in _CACHE:
        _CACHE["nc"] = _build()
    nc = _CACHE["nc"]

    in_maps = make_in_maps(x, Wq, bq, Wk, bk, Wv, Wo)
    res = bass_utils.run_bass_kernel_spmd(nc, in_maps, core_ids=list(range(8)))

    out = np.zeros((B, S, D), dtype=np.float32)
    for c in range(8):
        out[c // 4] += res.results[c]["out_t"].T
    out += bo + bv @ Wo
    return out


# revision 5
# speedup vs baseline: 1.4730x; 1.2039x over previous
"""Multi-head attention (B=2, S=2048, D=1024, H=16, dk=64) on 8 Trainium2
NeuronCores via Bass/Tile.

Sharding: core c handles batch b = c//4 and head-group g = c%4 (4 heads,
256 qkv columns).  Each core computes its QKV projection slices, 4 heads of
attention, and a partial output projection against its 256-row slice of Wo.
The host sums the 4 partial outputs per batch (row-sharded Wo => partial
sums) and folds in the biases bo and bv@Wo (softmax rows sum to 1, so the
V-bias contributes exactly bv@Wo per token).

v3 design notes (vs v2 baseline at 378us):
- All matmuls bf16 (fp32 HIGH mode triggered the activity power throttle:
  50% PE util limit for 60% of runtime; bf16 runs 1 cyc/row at any free
  size).  PSUM accumulation stays fp32; softmax denominators stay fp32
  through the reciprocal path; num/denom share the same bf16 exp values so
  normalization error largely cancels.
- Host pre-transposes x and pre-casts/pre-arranges all weights into the
  exact SBUF layouts, removing the on-device DMA transposes, hi/lo split
  adds and fp32->f32r casts that serialized the first 42us.
- Weight DMAs ride the scalar-engine HWDGE queue, x/out DMAs the sync
  queue, so startup transfers overlap.
- Scores land in one shared 4-bank PSUM tensor [128, 4, 512]; ONE ACT exp
  per 2 k-chunks covers [128, 1024] and writes bf16 directly (AV rhs).
- AV lhsT = [V_h | 1] so PSUM row 64 accumulates the softmax denominators.
- Normalization: DVE reciprocal of the fp32 sums row, PE ones
  outer-product broadcast (f32r, N=512 => full rate), DVE multiply to
  bf16, then a partition-shifting SBUF->SBUF DMA routes heads into O^T.
"""

import numpy as np

P = 128
B, S, D = 2, 2048, 1024
H, DK = 16, 64
COLS = 256          # qkv columns per core (4 heads)
KC = D // P         # 8 contraction chunks for the projections
TT = 512            # token block (matmul free dim)
NJ = S // TT        # 4 token blocks
NT = S // P         # 16 token tiles
NKT = S // P        # 16 key tiles
VW = 65             # per-head AV lhsT width: ones column + 64 v-dims

_CACHE = {}


def _build():
    import concourse.bass as bass
    import concourse.tile as tile
    from concourse import bacc, mybir

    f32 = mybir.dt.float32
    f32r = mybir.dt.float32r
    bf16 = mybir.dt.bfloat16
    Exp = mybir.ActivationFunctionType.Exp

    nc = bacc.Bacc(
        "TRN2", target_bir_lowering=False, debug=False,
        enable_asserts=False, num_devices=8,
    )
    xt_d = nc.dram_tensor("xt", [P, KC, S], bf16, kind="ExternalInput").ap()
    wq_d = nc.dram_tensor("wq", [P, KC, COLS], bf16, kind="ExternalInput").ap()
    wk_d = nc.dram_tensor("wk", [P, KC, COLS], bf16, kind="ExternalInput").ap()
    wv_d = nc.dram_tensor("wv", [P, KC, COLS], bf16, kind="ExternalInput").ap()
    wo_d = nc.dram_tensor("wo", [P, 2, D], bf16, kind="ExternalInput").ap()
    bq_d = nc.dram_tensor("bq", [P, 2], f32, kind="ExternalInput").ap()
    bk_d = nc.dram_tensor("bk", [P, 2], f32, kind="ExternalInput").ap()
    out_d = nc.dram_tensor("out_t", [D, S], f32, kind="ExternalOutput").ap()

    with tile.TileContext(nc) as tc:
        with (
            tc.tile_pool(name="const", bufs=1) as const,
            tc.tile_pool(name="wpool", bufs=1) as wpool,
            tc.tile_pool(name="persist", bufs=1) as persist,
            tc.tile_pool(name="xtp", bufs=2) as xtp,
            tc.tile_pool(name="exps", bufs=3) as exps,
            tc.tile_pool(name="stage", bufs=3) as stage,
            tc.tile_pool(name="outst", bufs=4) as outst,
            tc.tile_pool(name="ps_sc", bufs=3, space="PSUM") as ps_sc,
            tc.tile_pool(name="ps_acc", bufs=2, space="PSUM") as ps_acc,
        ):
            ones_bf = const.tile([P, VW], bf16, tag="ones_bf")
            nc.vector.memset(ones_bf[:], 1.0)
            ones_f = const.tile([1, 64], f32, tag="ones_f")
            nc.vector.memset(ones_f[:], 1.0)
            ones_r = const.tile([1, 64], f32r, tag="ones_r")
            nc.vector.tensor_copy(ones_r[:], ones_f[:])

            # weights arrive pre-arranged/pre-cast; scalar-engine DMA queue
            wq_sb = wpool.tile([P, KC, COLS], bf16, tag="w_q")
            nc.scalar.dma_start(wq_sb[:], wq_d)
            wk_sb = wpool.tile([P, KC, COLS], bf16, tag="w_k")
            nc.scalar.dma_start(wk_sb[:], wk_d)
            wv_sb = wpool.tile([P, KC, COLS], bf16, tag="w_v")
            nc.scalar.dma_start(wv_sb[:], wv_d)
            wo_sb = wpool.tile([P, 2, D], bf16, tag="w_o")
            nc.scalar.dma_start(wo_sb[:], wo_d)
            bq_sb = const.tile([P, 2], f32, tag="bq")
            nc.scalar.dma_start(bq_sb[:], bq_d)
            bk_sb = const.tile([P, 2], f32, tag="bk")
            nc.scalar.dma_start(bk_sb[:], bk_d)

            # persistent activations (all bf16)
            qT = persist.tile([P, 2, S], bf16, tag="qT")    # [qcol, tok]
            kT = persist.tile([P, 2, S], bf16, tag="kT")    # [kcol, tok]
            vt = persist.tile([P, NT, 4 * VW], bf16, tag="vt")  # [tok, h*(V|1)]
            oT = persist.tile([P, 2, S], bf16, tag="oT")    # [vdim, tok]

            # ones column (index 64 of each head's VW slice)
            vt_heads = vt[:].rearrange("p t (h c) -> p t h c", c=VW)
            nc.vector.tensor_copy(
                vt_heads[:, :, :, 64],
                ones_bf[:, :NT * 4].rearrange("p (t h) -> p t h", h=4),
            )

            # ---- phase 1: QKV projections, per token block ----
            for j in range(NJ):
                xT = xtp.tile([P, KC, TT], bf16, tag="xT")
                nc.sync.dma_start(xT[:], xt_d[:, :, bass.ts(j, TT)])

                # Q^T, K^T: [qcol, tok] with bias
                for (wmat, bsb, dstT) in ((wq_sb, bq_sb, qT), (wk_sb, bk_sb, kT)):
                    for ct in range(2):
                        acc = ps_acc.tile([P, TT], f32, tag="acc", name="qk_acc")
                        for kc in range(KC):
                            nc.tensor.matmul(
                                acc[:], wmat[:, kc, bass.ts(ct, P)], xT[:, kc, :],
                                start=(kc == 0), stop=(kc == KC - 1),
                            )
                        nc.vector.tensor_scalar_add(
                            dstT[:, ct, bass.ts(j, TT)], acc[:], bsb[:, ct : ct + 1]
                        )

                # V: [tok, vcol]
                for ts in range(TT // P):
                    acc = ps_acc.tile([P, COLS], f32, tag="acc", name="v_acc")
                    for kc in range(KC):
                        nc.tensor.matmul(
                            acc[:], xT[:, kc, bass.ts(ts, P)], wv_sb[:, kc, :],
                            start=(kc == 0), stop=(kc == KC - 1),
                        )
                    tt = 4 * j + ts
                    nc.vector.tensor_copy(
                        vt_heads[:, tt, :, 0:64],
                        acc[:].rearrange("p (h c) -> p h c", c=64),
                    )

            # ---- phase 2 + 3 interleaved over token blocks ----
            for j in range(NJ):
                for p in range(2):
                    o_ps = [
                        ps_acc.tile([VW, TT], f32, tag="acc", name=f"o_ps{i}")
                        for i in range(2)
                    ]
                    # software-pipelined emission: scores run 2 k-chunks ahead,
                    # AV trails exp by one, so PE always has ready work while
                    # ACT's exp latency is in flight.  Each k-chunk's scores
                    # get their OWN double-buffered PSUM tile so the scheduler
                    # tracks exact deps (a single shared 4-bank tensor made
                    # every ACT wait on the latest scores matmul emitted
                    # before it, serializing ACT+scores at ~2us per chunk).
                    sc_tiles = {}

                    def sc_emit(kc):
                        sc = ps_sc.tile([P, 2, TT], f32, tag="sc", name="sc")
                        sc_tiles[kc] = sc
                        for i in range(2):
                            lo, hi = 64 * i, 64 * i + 64
                            nc.tensor.matmul(
                                sc[:, i, :],
                                kT[lo:hi, p, bass.ts(kc, P)],
                                qT[lo:hi, p, bass.ts(j, TT)],
                                start=True, stop=True,
                            )

                    def av_emit(kc, ex):
                        for i in range(2):
                            h = 2 * p + i
                            nc.tensor.matmul(
                                o_ps[i][:],
                                vt[:, kc, bass.ds(VW * h, VW)],
                                ex[:, i, :],
                                start=(kc == 0), stop=(kc == NKT - 1),
                            )

                    sc_emit(0)
                    sc_emit(1)
                    prev = None
                    for kc in range(NKT):
                        ex = exps.tile([P, 2, TT], bf16, tag="exp", name="ex")
                        nc.scalar.activation(
                            ex[:], sc_tiles.pop(kc)[:], Exp,
                            scale=0.125,
                        )
                        if prev is not None:
                            av_emit(kc - 1, prev)
                        if kc + 2 < NKT:
                            sc_emit(kc + 2)
                        prev = ex
                    av_emit(NKT - 1, prev)

                    # normalize both heads into O^T: fp32 denominators,
                    # PE ones outer-product broadcast, bf16 result
                    o32 = stage.tile([P, TT], bf16, tag="o32", name="o32")
                    for i in range(2):
                        osb = stage.tile([P, TT], bf16, tag="osb", name="osb")
                        nc.vector.tensor_copy(osb[0:VW, :], o_ps[i][:])
                        srow = stage.tile([1, TT], f32r, tag="srow", name="srow")
                        nc.vector.tensor_copy(srow[:], o_ps[i][64:65, :])
                        rbc = ps_acc.tile([64, TT], f32, tag="acc", name="rbc")
                        nc.tensor.matmul(
                            rbc[:], ones_r[0:1, :], srow[0:1, :],
                            start=True, stop=True,
                        )
                        rbs = stage.tile([64, TT], f32, tag="rbs", name="rbs")
                        nc.vector.reciprocal_approx_fast(rbs[:], rbc[:])
                        onrm = stage.tile([P, TT], bf16, tag="onrm", name="onrm")
                        nc.vector.tensor_tensor(
                            onrm[0:64, :], osb[0:64, :], rbs[:],
                            mybir.AluOpType.mult,
                        )
                        nc.sync.dma_start(
                            o32[bass.ds(64 * i, 64), :], onrm[0:64, :]
                        )
                    nc.vector.tensor_copy(oT[:, p, bass.ts(j, TT)], o32[:])

                # partial output projection for this token block
                for oc in range(D // P):
                    acc = ps_acc.tile([P, TT], f32, tag="acc", name="wo_acc")
                    for vc in range(2):
                        nc.tensor.matmul(
                            acc[:], wo_sb[:, vc, bass.ts(oc, P)],
                            oT[:, vc, bass.ts(j, TT)],
                            start=(vc == 0), stop=(vc == 1),
                        )
                    st = outst.tile([P, TT], f32, tag="outst", name="outst")
                    nc.vector.tensor_copy(st[:], acc[:])
                    nc.sync.dma_start(out_d[bass.ts(oc, P), bass.ts(j, TT)], st[:])

    nc.compile()
    return nc


def make_in_maps(x, Wq, bq, Wk, bk, Wv, Wo):
    import ml_dtypes

    bf = ml_dtypes.bfloat16

    def shard3(a, o):  # [o*P, f] -> [P, o, f]
        return np.ascontiguousarray(
            a.reshape(o, P, a.shape[1]).transpose(1, 0, 2)
        )

    xt = [
        shard3(np.ascontiguousarray(x[b].T).astype(bf), KC) for b in range(B)
    ]

    in_maps = []
    for c in range(8):
        b, g = divmod(c, 4)
        cs = slice(COLS * g, COLS * (g + 1))
        in_maps.append({
            "xt": xt[b],
            "wq": shard3(Wq[:, cs].astype(bf), KC),
            "wk": shard3(Wk[:, cs].astype(bf), KC),
            "wv": shard3(Wv[:, cs].astype(bf), KC),
            "wo": shard3(Wo[cs, :].astype(bf), 2),
            "bq": np.ascontiguousarray(bq[cs].reshape(2, P).T),
            "bk": np.ascontiguousarray(bk[cs].reshape(2, P).T),
        })
    return in_maps


def kernel(x, Wq, bq, Wk, bk, Wv, bv, Wo, bo):
    from concourse import bass_utils

    x = np.asarray(x, dtype=np.float32)
    Wq = np.asarray(Wq, dtype=np.float32)
    Wk = np.asarray(Wk, dtype=np.float32)
    Wv = np.asarray(Wv, dtype=np.float32)
    Wo = np.asarray(Wo, dtype=np.float32)
    bq = np.asarray(bq, dtype=np.float32)
    bk = np.asarray(bk, dtype=np.float32)
    bv = np.asarray(bv, dtype=np.float32)
    bo = np.asarray(bo, dtype=np.float32)

    if "nc" not in _CACHE:
        _CACHE["nc"] = _build()
    nc = _CACHE["nc"]

    in_maps = make_in_maps(x, Wq, bq, Wk, bk, Wv, Wo)
    res = bass_utils.run_bass_kernel_spmd(nc, in_maps, core_ids=list(range(8)))

    out = np.zeros((B, S, D), dtype=np.float32)
    for c in range(8):
        out[c // 4] += res.results[c]["out_t"].T
    out += bo + bv @ Wo
    return out


# revision 6
# speedup vs baseline: 1.7548x; 1.1914x over previous
"""Multi-head attention (B=2, S=2048, D=1024, H=16, dk=64) on 8 Trainium2
NeuronCores via Bass/Tile.

Sharding: core c handles batch b = c//4 and head-group g = c%4 (4 heads,
256 qkv columns).  Each core computes its QKV projection slices, 4 heads of
attention, and a partial output projection against its 256-row slice of Wo.
The host sums the 4 partial outputs per batch (row-sharded Wo => partial
sums) and folds in the biases bo and bv@Wo (softmax rows sum to 1, so the
V-bias contributes exactly bv@Wo per token).

v6 design notes (baseline 378us -> v5 263us -> this):
- All matmuls bf16 (fp32 HIGH mode triggers the PE activity power
  throttle); PSUM accumulation fp32; softmax denominators fp32 through the
  reciprocal; num/denom share the same bf16 exp values so normalization
  error largely cancels.  Host pre-transposes x / pre-arranges weights.
- Phase 2 is ACT(exp)-bound (~1.16us per k-chunk of [128,2x512]).  Scores
  use per-chunk PSUM tiles from a 3-deep pool (6 banks) so the exp stream
  never waits PSUM WAR; AV accumulators [65,512]x2 own the last 2 banks.
  Projection/outproj/broadcast accumulators BORROW slots of the score pool
  (same tag) since all 8 banks are committed.
- Fully software-pipelined single pass: K/V/Q projections, the previous
  group's normalize (split: free-the-accumulator copies at group open, PE
  broadcast+multiply deferred to kc 2), and the previous block's output
  projection (one chunk per kc) are all interleaved into the attention
  groups' k-chunk loops, keeping PE busy under the exp stream and the exp
  stream free of group-boundary bubbles.
"""

import numpy as np

P = 128
B, S, D = 2, 2048, 1024
H, DK = 16, 64
COLS = 256          # qkv columns per core (4 heads)
KC = D // P         # 8 contraction chunks for the projections
TT = 512            # token block (matmul free dim)
NJ = S // TT        # 4 token blocks
NT = S // P         # 16 token tiles
NKT = S // P        # 16 key tiles
VW = 65             # per-head AV lhsT width: ones column + 64 v-dims

_CACHE = {}


def _build():
    import concourse.bass as bass
    import concourse.tile as tile
    from concourse import bacc, mybir

    f32 = mybir.dt.float32
    f32r = mybir.dt.float32r
    bf16 = mybir.dt.bfloat16
    Exp = mybir.ActivationFunctionType.Exp

    nc = bacc.Bacc(
        "TRN2", target_bir_lowering=False, debug=False,
        enable_asserts=False, num_devices=8,
    )
    xt_d = nc.dram_tensor("xt", [P, KC, S], bf16, kind="ExternalInput").ap()
    wq_d = nc.dram_tensor("wq", [P, KC, COLS], bf16, kind="ExternalInput").ap()
    wk_d = nc.dram_tensor("wk", [P, KC, COLS], bf16, kind="ExternalInput").ap()
    wv_d = nc.dram_tensor("wv", [P, KC, COLS], bf16, kind="ExternalInput").ap()
    wo_d = nc.dram_tensor("wo", [P, 2, D], bf16, kind="ExternalInput").ap()
    bq_d = nc.dram_tensor("bq", [P, 2], f32, kind="ExternalInput").ap()
    bk_d = nc.dram_tensor("bk", [P, 2], f32, kind="ExternalInput").ap()
    out_d = nc.dram_tensor("out_t", [D, S], f32, kind="ExternalOutput").ap()

    with tile.TileContext(nc) as tc:
        with (
            tc.tile_pool(name="const", bufs=1) as const,
            tc.tile_pool(name="wpool", bufs=1) as wpool,
            tc.tile_pool(name="persist", bufs=1) as persist,
            tc.tile_pool(name="exps", bufs=3) as exps,
            tc.tile_pool(name="stage", bufs=3) as stage,
            tc.tile_pool(name="outst", bufs=4) as outst,
            tc.tile_pool(name="ps_sc", bufs=3, space="PSUM") as ps_sc,
            tc.tile_pool(name="ps_acc", bufs=2, space="PSUM") as ps_acc,
        ):
            ones_bf = const.tile([P, VW], bf16, tag="ones_bf")
            nc.vector.memset(ones_bf[:], 1.0)
            ones_f = const.tile([1, 64], f32, tag="ones_f")
            nc.vector.memset(ones_f[:], 1.0)
            ones_r = const.tile([1, 64], f32r, tag="ones_r")
            nc.vector.tensor_copy(ones_r[:], ones_f[:])

            # weights pre-arranged/pre-cast by the host; scalar-engine DMA
            # queue, ordered by first use (K proj is first)
            wk_sb = wpool.tile([P, KC, COLS], bf16, tag="w_k")
            nc.scalar.dma_start(wk_sb[:], wk_d)
            bk_sb = const.tile([P, 2], f32, tag="bk")
            nc.scalar.dma_start(bk_sb[:], bk_d)
            wv_sb = wpool.tile([P, KC, COLS], bf16, tag="w_v")
            nc.scalar.dma_start(wv_sb[:], wv_d)
            wq_sb = wpool.tile([P, KC, COLS], bf16, tag="w_q")
            nc.scalar.dma_start(wq_sb[:], wq_d)
            bq_sb = const.tile([P, 2], f32, tag="bq")
            nc.scalar.dma_start(bq_sb[:], bq_d)
            wo_sb = wpool.tile([P, 2, D], bf16, tag="w_o")
            nc.scalar.dma_start(wo_sb[:], wo_d)

            # persistent activations (all bf16)
            xall = persist.tile([P, KC, S], bf16, tag="xall")  # x^T, all blocks
            qT = persist.tile([P, 2, S], bf16, tag="qT")    # [qcol, tok]
            kT = persist.tile([P, 2, S], bf16, tag="kT")    # [kcol, tok]
            vt = persist.tile([P, NT, 4 * VW], bf16, tag="vt")  # [tok, h*(V|1)]
            oT = persist.tile([P, 2, S], bf16, tag="oT")    # [vdim, tok]

            for j in range(NJ):
                nc.sync.dma_start(
                    xall[:, :, bass.ts(j, TT)], xt_d[:, :, bass.ts(j, TT)]
                )

            # ones column (index 64 of each head's VW slice)
            vt_heads = vt[:].rearrange("p t (h c) -> p t h c", c=VW)
            nc.vector.tensor_copy(
                vt_heads[:, :, :, 64],
                ones_bf[:, :NT * 4].rearrange("p (t h) -> p t h", h=4),
            )

            # ---- projection emitters (accumulators borrow score-pool slots)
            def proj_qk(wmat, bsb, dstT, ct, j):
                acc = ps_sc.tile([P, TT], f32, tag="sc", name="qk_acc")
                for kc in range(KC):
                    nc.tensor.matmul(
                        acc[:], wmat[:, kc, bass.ts(ct, P)],
                        xall[:, kc, bass.ts(j, TT)],
                        start=(kc == 0), stop=(kc == KC - 1),
                    )
                nc.vector.tensor_scalar_add(
                    dstT[:, ct, bass.ts(j, TT)], acc[:], bsb[:, ct : ct + 1]
                )

            def proj_v(j, ts_):
                acc = ps_sc.tile([P, COLS], f32, tag="sc", name="v_acc")
                for kc in range(KC):
                    nc.tensor.matmul(
                        acc[:], xall[:, kc, bass.ds(j * TT + ts_ * P, P)],
                        wv_sb[:, kc, :],
                        start=(kc == 0), stop=(kc == KC - 1),
                    )
                nc.vector.tensor_copy(
                    vt_heads[:, 4 * j + ts_, :, 0:64],
                    acc[:].rearrange("p (h c) -> p h c", c=64),
                )

            def outproj_chunk(j, oc):
                acc = ps_sc.tile([P, TT], f32, tag="sc", name="wo_acc")
                for vc in range(2):
                    nc.tensor.matmul(
                        acc[:], wo_sb[:, vc, bass.ts(oc, P)],
                        oT[:, vc, bass.ts(j, TT)],
                        start=(vc == 0), stop=(vc == 1),
                    )
                st = outst.tile([P, TT], f32, tag="outst", name="outst")
                nc.vector.tensor_copy(st[:], acc[:])
                nc.sync.dma_start(out_d[bass.ts(oc, P), bass.ts(j, TT)], st[:])

            # ---- two-part normalize: A frees the AV accumulators, B does
            # the PE broadcast + multiply + partition-shift into O^T
            def norm_partA(o_ps):
                parts = []
                for i in range(2):
                    osb = stage.tile([P, TT], bf16, tag="osb", name="osb")
                    nc.vector.tensor_copy(osb[0:VW, :], o_ps[i][:])
                    srow = stage.tile([1, TT], f32r, tag="srow", name="srow")
                    nc.vector.tensor_copy(srow[:], o_ps[i][64:65, :])
                    parts.append((osb, srow))
                return parts

            def norm_partB(parts, j, p):
                o32 = stage.tile([P, TT], bf16, tag="o32", name="o32")
                for i in range(2):
                    osb, srow = parts[i]
                    rbc = ps_sc.tile([64, TT], f32, tag="sc", name="rbc")
                    nc.tensor.matmul(
                        rbc[:], ones_r[0:1, :], srow[0:1, :],
                        start=True, stop=True,
                    )
                    rbs = stage.tile([64, TT], f32, tag="rbs", name="rbs")
                    nc.vector.reciprocal_approx_fast(rbs[:], rbc[:])
                    onrm = stage.tile([P, TT], bf16, tag="onrm", name="onrm")
                    nc.vector.tensor_tensor(
                        onrm[0:64, :], osb[0:64, :], rbs[:],
                        mybir.AluOpType.mult,
                    )
                    nc.sync.dma_start(o32[bass.ds(64 * i, 64), :], onrm[0:64, :])
                nc.vector.tensor_copy(oT[:, p, bass.ts(j, TT)], o32[:])

            # ---- deferred-work schedule per (j, p) group ----
            # inserts[g][kc] -> emitters to run after that kc's sc/av/ACT
            groups = [(j, p) for j in range(NJ) for p in range(2)]
            inserts = {g: {} for g in groups}

            def add(g, kc, fn):
                inserts[g].setdefault(kc, []).append(fn)

            def K(ct, j):
                return lambda: proj_qk(wk_sb, bk_sb, kT, ct, j)

            def Q(ct, j):
                return lambda: proj_qk(wq_sb, bq_sb, qT, ct, j)

            def V(j, ts_):
                return lambda: proj_v(j, ts_)

            # group (0,0): feed K(ct0)/V progressively for later blocks
            for b in range(1, NJ):
                add((0, 0), 4 * b - 3, K(0, b))
                for t in range(4):
                    add((0, 0), 4 * b - 2 + t, V(b, t))
            add((0, 0), 14, K(1, 0))
            add((0, 0), 15, Q(1, 0))
            # group (0,1): feed K(ct1) for later blocks; Q(ct0, j1) late
            for b in range(1, NJ):
                add((0, 1), 4 * b - 3, K(1, b))
            add((0, 1), 13, Q(0, 1))
            # steady state: Q for the next groups
            for j in range(1, NJ):
                add((j, 0), 5, Q(1, j))
                if j + 1 < NJ:
                    add((j, 1), 8, Q(0, j + 1))
            # outproj of block j-1 spread over (j, 0)'s kc 8..15
            for j in range(1, NJ):
                for oc in range(D // P):
                    add((j, 0), 8 + oc, lambda j=j, oc=oc: outproj_chunk(j - 1, oc))

            # prime: minimum work before the first attention group
            proj_qk(wk_sb, bk_sb, kT, 0, 0)
            proj_qk(wq_sb, bq_sb, qT, 0, 0)
            for t in range(4):
                proj_v(0, t)

            # ---- the pipelined attention groups ----
            pending_normA = None  # partA output of the previous group
            pending_norm_jp = None

            for (j, p) in groups:
                # free the previous group's AV accumulators first
                partB_args = None
                if pending_normA is not None:
                    parts = pending_normA()
                    partB_args = (parts, pending_norm_jp[0], pending_norm_jp[1])

                o_ps = [
                    ps_acc.tile([VW, TT], f32, tag="acc", name=f"o_ps{i}")
                    for i in range(2)
                ]
                sc_tiles = {}

                def sc_emit(kc, j=j, p=p, sc_tiles=sc_tiles):
                    sc = ps_sc.tile([P, 2, TT], f32, tag="sc", name="sc")
                    sc_tiles[kc] = sc
                    for i in range(2):
                        lo, hi = 64 * i, 64 * i + 64
                        nc.tensor.matmul(
                            sc[:, i, :],
                            kT[lo:hi, p, bass.ts(kc, P)],
                            qT[lo:hi, p, bass.ts(j, TT)],
                            start=True, stop=True,
                        )

                def av_emit(kc, ex, j=j, p=p, o_ps=o_ps):
                    for i in range(2):
                        h = 2 * p + i
                        nc.tensor.matmul(
                            o_ps[i][:],
                            vt[:, kc, bass.ds(VW * h, VW)],
                            ex[:, i, :],
                            start=(kc == 0), stop=(kc == NKT - 1),
                        )

                sc_emit(0)
                sc_emit(1)
                ins = inserts[(j, p)]
                prev = None
                for kc in range(NKT):
                    ex = exps.tile([P, 2, TT], bf16, tag="exp", name="ex")
                    nc.scalar.activation(
                        ex[:], sc_tiles.pop(kc)[:], Exp, scale=0.125,
                    )
                    if prev is not None:
                        av_emit(kc - 1, prev)
                    if kc + 2 < NKT:
                        sc_emit(kc + 2)
                    if kc == 2 and partB_args is not None:
                        norm_partB(*partB_args)
                        partB_args = None
                    for fn in ins.get(kc, ()):
                        fn()
                    prev = ex
                av_emit(NKT - 1, prev)

                pending_normA = (lambda o_ps=o_ps: norm_partA(o_ps))
                pending_norm_jp = (j, p)

            # tail: last group's normalize + last block's output projection
            parts = pending_normA()
            norm_partB(parts, NJ - 1, 1)
            for oc in range(D // P):
                outproj_chunk(NJ - 1, oc)

    nc.compile()
    return nc


def make_in_maps(x, Wq, bq, Wk, bk, Wv, Wo):
    import ml_dtypes

    bf = ml_dtypes.bfloat16

    def shard3(a, o):  # [o*P, f] -> [P, o, f]
        return np.ascontiguousarray(
            a.reshape(o, P, a.shape[1]).transpose(1, 0, 2)
        )

    xt = [
        shard3(np.ascontiguousarray(x[b].T).astype(bf), KC) for b in range(B)
    ]

    in_maps = []
    for c in range(8):
        b, g = divmod(c, 4)
        cs = slice(COLS * g, COLS * (g + 1))
        in_maps.append({
            "xt": xt[b],
            "wq": shard3(Wq[:, cs].astype(bf), KC),
            "wk": shard3(Wk[:, cs].astype(bf), KC),
            "wv": shard3(Wv[:, cs].astype(bf), KC),
            "wo": shard3(Wo[cs, :].astype(bf), 2),
            "bq": np.ascontiguousarray(bq[cs].reshape(2, P).T),
            "bk": np.ascontiguousarray(bk[cs].reshape(2, P).T),
        })
    return in_maps


def kernel(x, Wq, bq, Wk, bk, Wv, bv, Wo, bo):
    from concourse import bass_utils

    x = np.asarray(x, dtype=np.float32)
    Wq = np.asarray(Wq, dtype=np.float32)
    Wk = np.asarray(Wk, dtype=np.float32)
    Wv = np.asarray(Wv, dtype=np.float32)
    Wo = np.asarray(Wo, dtype=np.float32)
    bq = np.asarray(bq, dtype=np.float32)
    bk = np.asarray(bk, dtype=np.float32)
    bv = np.asarray(bv, dtype=np.float32)
    bo = np.asarray(bo, dtype=np.float32)

    if "nc" not in _CACHE:
        _CACHE["nc"] = _build()
    nc = _CACHE["nc"]

    in_maps = make_in_maps(x, Wq, bq, Wk, bk, Wv, Wo)
    res = bass_utils.run_bass_kernel_spmd(nc, in_maps, core_ids=list(range(8)))

    out = np.zeros((B, S, D), dtype=np.float32)
    for c in range(8):
        out[c // 4] += res.results[c]["out_t"].T
    out += bo + bv @ Wo
    return out
